# Initial kernel scaffold
#
"""Distributed EnhancedResGCN forward for 8 Trainium2 NeuronCores.

Strategy (graph/data parallel over nodes, per the sharding hint):
  - nodes are sharded contiguously across 8 cores; all dense per-node math is
    done on the owning core with replicated weights.
  - each SpMM (7 total: 1 conv for layer 0 via the projection trick, then
    neighbor-mean + conv-delta for layers 1..3) is done dst-side: the needed
    per-node feature tables are AllGathered (node rows, [N,64] or [N,128]),
    then each core gathers rows for its own edges with dma_gather and reduces
    them into per-128-node-block PSUM accumulators with one-hot matmuls
    (edges sorted by destination block on the host).
  - BatchNorm statistics are computed from per-shard second-moment matmuls and
    combined with a tiny AllReduce.

Host-side preprocessing (numpy) builds the edge tiling: edges grouped by
(dst block, src bucket of 25000 so gather indices fit int16), padded to
128-edge tiles with a structure that is identical across all 8 cores (SPMD:
one program, per-core data).
"""

import os
import sys

for _p in ("/opt/trn_rl_repo", "/root/.axon_site/_ro/trn_rl_repo"):
    if os.path.isdir(_p) and _p not in sys.path:
        sys.path.append(_p)

import numpy as np

import concourse.bass as bass
import concourse.tile as tile
from concourse import bacc, mybir
from concourse.bass_utils import run_bass_kernel_spmd

try:
    import bass_rust
except ImportError:  # pragma: no cover
    bass_rust = None

F32 = mybir.dt.float32
I16 = mybir.dt.int16
AF = mybir.ActivationFunctionType
OP = mybir.AluOpType

NCORES = 8
BLK = 128
BUCKET = 25000
MAX_PHASE_TILES = 36
EPS = 1e-5
STAGE = int(os.environ.get("GCN_STAGE", "99"))
SPMM_MODE = os.environ.get("GCN_SPMM", "full")  # gather | mm | full


# --------------------------------------------------------------------------
# walrus in this container rejects instructions carrying >1 sem wait; hoist
# extras onto same-engine NOPs inserted right before the instruction.
def _split_excess_waits(nc, max_waits=1):
    n_split = 0
    for fn in nc.m.functions:
        for blk in fn.blocks:
            insts = blk.instructions
            pos = 0
            while pos < len(insts):
                inst = insts[pos]
                si = inst.sync_info
                waits = list(si.on_wait) if si is not None and si.on_wait else []
                if len(waits) > max_waits:
                    si.on_wait = waits[:max_waits]
                    extra = waits[max_waits:]
                    at = pos
                    for j in range(0, len(extra), max_waits):
                        nop = mybir.InstNoOp(
                            name=f"waitnop_{n_split}_{j}", ins=[], outs=[]
                        )
                        nop.engine = inst.engine
                        nop.sync_info = bass_rust.SyncInfo(
                            on_wait=extra[j : j + max_waits], on_update=[]
                        )
                        try:
                            nc.register_instruction(nop, overwrite=True)
                        except Exception:
                            pass
                        insts.insert(at, nop)
                        at += 1
                        pos += 1
                    n_split += 1
                pos += 1
    return n_split


# --------------------------------------------------------------------------
# host-side edge preprocessing
def _preprocess(N, src, dst):
    S = N // NCORES
    NBLK = (S + BLK - 1) // BLK
    nbuck = (N + BUCKET - 1) // BUCKET
    bucket_sizes = [min(BUCKET, N - j * BUCKET) for j in range(nbuck)]

    in_deg = np.bincount(dst, minlength=N).astype(np.float64)
    out_deg = np.bincount(src, minlength=N).astype(np.float64)
    in_deg_c = np.maximum(in_deg, 1.0)
    out_deg_c = np.maximum(out_deg, 1.0)
    in_norm = (in_deg_c**-0.5).astype(np.float32)
    out_norm = (out_deg_c**-0.5).astype(np.float32)
    r_indeg = (1.0 / in_deg_c).astype(np.float32)

    # per-core sorted edges + group boundaries on the (block, bucket) key
    percore = []
    counts = np.zeros((NCORES, NBLK * nbuck), np.int64)
    for c in range(NCORES):
        m = (dst >= c * S) & (dst < (c + 1) * S)
        es = src[m]
        ed = dst[m] - c * S
        bl = ed // BLK
        bu = es // BUCKET
        key = bl * nbuck + bu
        order = np.lexsort((es, key))
        es, ed, key = es[order], ed[order], key[order]
        starts = np.searchsorted(key, np.arange(NBLK * nbuck))
        ends = np.searchsorted(key, np.arange(NBLK * nbuck) + 1)
        counts[c] = ends - starts
        percore.append((es, ed, starts, ends))

    maxc = counts.max(axis=0).reshape(NBLK, nbuck)
    tiles_per = (maxc + BLK - 1) // BLK  # [NBLK, nbuck]
    # ensure every block has at least one tile so PSUM accum groups exist
    empty = tiles_per.sum(axis=1) == 0
    tiles_per[empty, 0] = 1

    # phase packing: whole blocks, limited total tiles
    blk_tiles = tiles_per.sum(axis=1)
    phases = []
    cur, cur_t = [], 0
    for b in range(NBLK):
        if cur and cur_t + blk_tiles[b] > MAX_PHASE_TILES:
            phases.append(cur)
            cur, cur_t = [], 0
        cur.append(b)
        cur_t += blk_tiles[b]
    if cur:
        phases.append(cur)

    # global tile order: phase -> bucket -> block
    phase_meta = []
    T = 0
    for blist in phases:
        calls = []
        pcol = 0
        block_tiles = {b: [] for b in blist}
        for j in range(nbuck):
            cnt = int(sum(tiles_per[b, j] for b in blist))
            if cnt == 0:
                continue
            calls.append(dict(bucket=j, off=pcol, cnt=cnt, gtile=T))
            for b in blist:
                for _ in range(int(tiles_per[b, j])):
                    block_tiles[b].append((pcol, T))
                    pcol += 1
                    T += 1
        phase_meta.append(
            dict(
                blocks=[dict(b=b, tiles=block_tiles[b]) for b in blist],
                ntiles=pcol,
                calls=calls,
            )
        )

    # per-core idx/seg arrays in the same global order
    idx_all = np.zeros((NCORES, T, 128), np.int16)
    seg_all = np.full((NCORES, T, 128), 255.0, np.float32)
    for c in range(NCORES):
        es, ed, starts, ends = percore[c]
        t_cursor = 0
        for ph in phase_meta:
            for call in ph["calls"]:
                j = call["bucket"]
                for binfo in ph["blocks"]:
                    b = binfo["b"]
                    nt = int(tiles_per[b, j])
                    if nt == 0:
                        continue
                    g = b * nbuck + j
                    s, e = int(starts[g]), int(ends[g])
                    cnt = e - s
                    loc_idx = (es[s:e] - j * BUCKET).astype(np.int16)
                    loc_seg = (ed[s:e] - b * BLK).astype(np.float32)
                    pad_idx = loc_idx[-1] if cnt > 0 else np.int16(0)
                    block_idx = np.full(nt * 128, pad_idx, np.int16)
                    block_seg = np.full(nt * 128, 255.0, np.float32)
                    block_idx[:cnt] = loc_idx
                    block_seg[:cnt] = loc_seg
                    idx_all[c, t_cursor : t_cursor + nt] = block_idx.reshape(nt, 128)
                    seg_all[c, t_cursor : t_cursor + nt] = block_seg.reshape(nt, 128)
                    t_cursor += nt
        assert t_cursor == T

    # wrap idx: per tile [128] -> [16, 8] (pos i -> [i%16, i//16]), concat along
    # cols, replicate to 128 partitions.
    idx_wrapped = np.zeros((NCORES, 128, T * 8), np.int16)
    w = idx_all.reshape(NCORES, T, 8, 16).transpose(0, 3, 1, 2).reshape(NCORES, 16, T * 8)
    for r in range(8):
        idx_wrapped[:, 16 * r : 16 * (r + 1), :] = w
    seg_cols = seg_all.transpose(0, 2, 1).copy()  # [NCORES, 128, T]

    def block_major(vec):  # [S] -> [128, NBLK], zero-padded
        out = np.zeros((NCORES, 128, NBLK), np.float32)
        for c in range(NCORES):
            v = vec[c * S : (c + 1) * S]
            pad = np.zeros(NBLK * BLK, np.float32)
            pad[:S] = v
            out[c] = pad.reshape(NBLK, BLK).T
        return out

    return dict(
        N=N,
        S=S,
        NBLK=NBLK,
        nbuck=nbuck,
        bucket_sizes=bucket_sizes,
        phases=phase_meta,
        T=T,
        idx=idx_wrapped,
        seg=seg_cols,
        in_norm_b=block_major(in_norm),
        out_norm_b=block_major(out_norm),
        r_indeg_b=block_major(r_indeg),
    )


# --------------------------------------------------------------------------
def _build_program(meta, IN, H, C, L, reps=1):
    N, S, NBLK = meta["N"], meta["S"], meta["NBLK"]
    T = meta["T"]
    H2 = 2 * H
    n_inv = 1.0 / N

    nc = bacc.Bacc("TRN2", target_bir_lowering=False, debug=False, num_devices=NCORES)

    # ---- I/O ----
    featT_d = nc.dram_tensor("featT", [IN, S], F32, kind="ExternalInput")
    idx_d = nc.dram_tensor("idx", [128, T * 8], I16, kind="ExternalInput")
    seg_d = nc.dram_tensor("seg", [128, T], F32, kind="ExternalInput")
    innorm_d = nc.dram_tensor("innorm", [128, NBLK], F32, kind="ExternalInput")
    outnorm_d = nc.dram_tensor("outnorm", [128, NBLK], F32, kind="ExternalInput")
    rindeg_d = nc.dram_tensor("rindeg", [128, NBLK], F32, kind="ExternalInput")
    iota_d = nc.dram_tensor("iota", [128, 128], F32, kind="ExternalInput")
    ident_d = nc.dram_tensor("ident", [128, 128], F32, kind="ExternalInput")
    ones_d = nc.dram_tensor("ones", [128, 1], F32, kind="ExternalInput")
    wenc1_d = nc.dram_tensor("wenc1", [IN, IN // 2], F32, kind="ExternalInput")
    benc1_d = nc.dram_tensor("benc1", [IN // 2, 1], F32, kind="ExternalInput")
    wenc2_d = nc.dram_tensor("wenc2", [IN // 2, IN // 4], F32, kind="ExternalInput")
    benc2_d = nc.dram_tensor("benc2", [IN // 4, 1], F32, kind="ExternalInput")
    wenc3_d = nc.dram_tensor("wenc3", [IN // 4, IN], F32, kind="ExternalInput")
    benc3_d = nc.dram_tensor("benc3", [IN, 1], F32, kind="ExternalInput")
    w0_d = nc.dram_tensor("w0", [IN, H], F32, kind="ExternalInput")
    b_d = nc.dram_tensor("bvec", [H, L], F32, kind="ExternalInput")  # unused cols kept for uniform I/O
    wr_d = nc.dram_tensor("wrest", [H, (L - 1) * H], F32, kind="ExternalInput")
    gam_d = nc.dram_tensor("gam", [H, L], F32, kind="ExternalInput")
    bet_d = nc.dram_tensor("bet", [H, L], F32, kind="ExternalInput")
    watt1_d = nc.dram_tensor("watt1", [H2, H], F32, kind="ExternalInput")
    batt1_d = nc.dram_tensor("batt1", [H, 1], F32, kind="ExternalInput")
    watt2_d = nc.dram_tensor("watt2", [H, 1], F32, kind="ExternalInput")
    batt2_d = nc.dram_tensor("batt2", [1, 1], F32, kind="ExternalInput")
    wfc_d = nc.dram_tensor("wfc", [H, C], F32, kind="ExternalInput")
    bfc_d = nc.dram_tensor("bfc", [C, 1], F32, kind="ExternalInput")
    outT_d = nc.dram_tensor("outT", [C, S], F32, kind="ExternalOutput")

    # internal DRAM
    P_local = nc.dram_tensor("P_local", [S, H2], F32)
    P_full = nc.dram_tensor("P_full", [N, H2], F32, addr_space=os.environ.get("GCN_AS", "Shared"))
    g_local = nc.dram_tensor("g_local", [S, H], F32)
    g_full = nc.dram_tensor("g_full", [N, H], F32, addr_space=os.environ.get("GCN_AS", "Shared"))
    st_local = nc.dram_tensor("st_local", [H, H + 2], F32)
    st_full = nc.dram_tensor("st_full", [H, H + 2], F32, addr_space=os.environ.get("GCN_AS", "Shared"))

    RG = [list(range(NCORES))]

    with tile.TileContext(nc) as tc:
        import contextlib

        ctx = contextlib.ExitStack()
        const = ctx.enter_context(tc.tile_pool(name="const", bufs=1))
        persist = ctx.enter_context(tc.tile_pool(name="persist", bufs=1))
        gath = ctx.enter_context(tc.tile_pool(name="gath", bufs=2))
        mpool = ctx.enter_context(tc.tile_pool(name="mpool", bufs=6))
        stage = ctx.enter_context(tc.tile_pool(name="stage", bufs=3))
        small = ctx.enter_context(tc.tile_pool(name="small", bufs=2))
        pmain = ctx.enter_context(tc.tile_pool(name="pmain", bufs=2, space="PSUM"))
        pstat = ctx.enter_context(tc.tile_pool(name="pstat", bufs=1, space="PSUM"))

        def load_const(dram, shape, dtype=F32, name=None):
            t = const.tile(shape, dtype, name=name or dram.name + "_s")
            nc.sync.dma_start(out=t[:], in_=dram[:])
            return t

        idx_s = load_const(idx_d, [128, T * 8], I16)
        seg_s = load_const(seg_d, [128, T])
        innorm = load_const(innorm_d, [128, NBLK])
        outnorm = load_const(outnorm_d, [128, NBLK])
        rindeg = load_const(rindeg_d, [128, NBLK])
        iota = load_const(iota_d, [128, 128])
        ident = load_const(ident_d, [128, 128])
        ones = load_const(ones_d, [128, 1])
        wenc1 = load_const(wenc1_d, [IN, IN // 2])
        benc1 = load_const(benc1_d, [IN // 2, 1])
        wenc2 = load_const(wenc2_d, [IN // 2, IN // 4])
        benc2 = load_const(benc2_d, [IN // 4, 1])
        wenc3 = load_const(wenc3_d, [IN // 4, IN])
        benc3 = load_const(benc3_d, [IN, 1])
        w0 = load_const(w0_d, [IN, H])
        bvec = load_const(b_d, [H, L])
        wrest = load_const(wr_d, [H, (L - 1) * H])
        gam = load_const(gam_d, [H, L])
        bet = load_const(bet_d, [H, L])
        watt1 = load_const(watt1_d, [H2, H])
        batt1 = load_const(batt1_d, [H, 1])
        watt2 = load_const(watt2_d, [H, 1])
        batt2 = load_const(batt2_d, [1, 1])
        wfc = load_const(wfc_d, [H, C])
        bfc = load_const(bfc_d, [C, 1])

        hbuf = persist.tile([128, NBLK * H], F32, name="hbuf")
        sonh = persist.tile([128, NBLK * H], F32, name="sonh")
        aggb = persist.tile([128, NBLK * H], F32, name="aggb")
        if SPMM_MODE != "full":
            nc.vector.memset(hbuf[:], 0.0)
            nc.vector.memset(sonh[:], 0.0)
            nc.vector.memset(aggb[:], 0.0)

        def bs_of(b):  # valid rows in block b
            return min(BLK, S - b * BLK)

        def spmm(table_ap, elem, epilogue, rep):
            """gather+scatter over all edges; epilogue(b, acc_psum) per block."""
            for pi, ph in enumerate(meta["phases"]):
                g = gath.tile([128, ph["ntiles"], elem], F32, tag="gath", name=f"g_{rep}_{elem}_{pi}")
                for call in ph["calls"]:
                    j = call["bucket"]
                    bsz = meta["bucket_sizes"][j]
                    off, cnt, gt = call["off"], call["cnt"], call["gtile"]
                    nc.gpsimd.dma_gather(
                        g[:, off : off + cnt, :],
                        table_ap[j * BUCKET : j * BUCKET + bsz, :],
                        idx_s[:, gt * 8 : (gt + cnt) * 8],
                        cnt * 128,
                        cnt * 128,
                        elem,
                        single_packet=False,
                    )
                if SPMM_MODE == "gather":
                    continue
                for binfo in ph["blocks"]:
                    b = binfo["b"]
                    acc = pmain.tile([128, elem], F32, tag="accum", name=f"acc{b}")
                    ntl = len(binfo["tiles"])
                    for k, (pcol, gt) in enumerate(binfo["tiles"]):
                        m = mpool.tile([128, 128], F32, tag="m", name=f"m{b}_{k}")
                        nc.vector.tensor_tensor(
                            out=m[:],
                            in0=iota[:],
                            in1=seg_s[:, gt : gt + 1].to_broadcast([128, 128]),
                            op=OP.is_equal,
                        )
                        nc.tensor.matmul(
                            out=acc[:],
                            lhsT=m[:],
                            rhs=g[:, pcol, :],
                            start=(k == 0),
                            stop=(k == ntl - 1),
                        )
                    if SPMM_MODE != "mm":
                        epilogue(b, acc)

        def transpose(src_ap, p, f, name):
            """src [p, f] (SBUF) -> psum tile [f, p]"""
            tp = pmain.tile([f, p], F32, tag="trans", space="PSUM", name=name)
            nc.tensor.transpose(out=tp[:], in_=src_ap, identity=ident[:p, :p])
            return tp

        def build_P(b, hn_ps, last_layer, rep):
            """write h_next (psum [128,H]) into hbuf + P_local rows"""
            bs = bs_of(b)
            hsl = hbuf[:, b * H : (b + 1) * H]
            nc.vector.tensor_copy(out=hsl, in_=hn_ps[:])
            if last_layer:
                return
            pst = stage.tile([128, H2], F32, tag="pst", name=f"pst{b}")
            nc.vector.tensor_copy(out=pst[:, :H], in_=hn_ps[:])
            nc.vector.tensor_scalar(
                out=pst[:, H:],
                in0=hn_ps[:],
                scalar1=outnorm[:, b : b + 1],
                scalar2=None,
                op0=OP.mult,
            )
            nc.sync.dma_start(
                out=P_local[b * BLK : b * BLK + bs, :], in_=pst[:bs, :]
            )

        def stats_to_scale_bias(layer, sum_ps, cov_ps, sumsq_ps, rep):
            """AllReduce moments -> scale [H,1], bias2 [H,1] (SBUF)."""
            li = str(layer) + "_" + str(rep)
            st_s = stage.tile([H, H + 2], F32, tag="stats", name=f"st{li}")
            if cov_ps is not None:
                nc.vector.tensor_copy(out=st_s[:, :H], in_=cov_ps[:])
            else:
                nc.vector.memset(st_s[:, :H], 0.0)
            nc.vector.tensor_copy(out=st_s[:, H : H + 1], in_=sum_ps[:])
            if sumsq_ps is not None:
                nc.vector.tensor_copy(out=st_s[:, H + 1 : H + 2], in_=sumsq_ps[:])
            else:
                nc.vector.memset(st_s[:, H + 1 : H + 2], 0.0)
            nc.sync.dma_start(out=st_local[:], in_=st_s[:])
            nc.gpsimd.collective_compute(
                "AllReduce",
                OP.add,
                replica_groups=RG,
                ins=[st_local[:]],
                outs=[st_full[:]],
            )
            stg = stage.tile([H, H + 2], F32, tag="stats2", name=f"stg{li}")
            nc.sync.dma_start(out=stg[:], in_=st_full[:])

            # gamma/beta are stored column-major [H, L]; the conv bias b
            # cancels inside BatchNorm and is not needed.
            gcol = gam[:, layer : layer + 1]
            betcol = bet[:, layer : layer + 1]

            mulin = small.tile([H, 1], F32, tag="mulin", name=f"ml{li}")
            e2n = small.tile([H, 1], F32, tag="e2n", name=f"e2{li}")
            if cov_ps is not None:
                # mulin = W.T @ (sum/N)
                msum = small.tile([H, 1], F32, tag="msum", name=f"ms{li}")
                nc.vector.tensor_scalar(
                    out=msum[:], in0=stg[:, H : H + 1], scalar1=n_inv, scalar2=None, op0=OP.mult
                )
                w_l = wrest[:, (layer - 1) * H : layer * H]
                ml_ps = pmain.tile([H, 1], F32, tag="mm", space="PSUM", name=f"mlp{li}")
                nc.tensor.matmul(out=ml_ps[:], lhsT=w_l, rhs=msum[:], start=True, stop=True)
                nc.vector.tensor_copy(out=mulin[:], in_=ml_ps[:])
                # E2 = diag(W.T Sigma W)/N
                a_ps = pmain.tile([H, H], F32, tag="mm", space="PSUM", name=f"ap{li}")
                nc.tensor.matmul(out=a_ps[:], lhsT=stg[:, :H], rhs=w_l, start=True, stop=True)
                bmat = stage.tile([H, H], F32, tag="bmat", name=f"bm{li}")
                nc.vector.tensor_tensor(out=bmat[:], in0=a_ps[:], in1=w_l, op=OP.mult)
                e2_ps = pmain.tile([H, 1], F32, tag="mm", space="PSUM", name=f"e2p{li}")
                nc.tensor.matmul(out=e2_ps[:], lhsT=bmat[:], rhs=ones[:H, :], start=True, stop=True)
                nc.vector.tensor_scalar(
                    out=e2n[:], in0=e2_ps[:], scalar1=n_inv, scalar2=None, op0=OP.mult
                )
            else:
                nc.vector.tensor_scalar(
                    out=mulin[:], in0=stg[:, H : H + 1], scalar1=n_inv, scalar2=None, op0=OP.mult
                )
                nc.vector.tensor_scalar(
                    out=e2n[:], in0=stg[:, H + 1 : H + 2], scalar1=n_inv, scalar2=None, op0=OP.mult
                )
            # var = e2n - mulin^2 ; rstd = sqrt(1/(var+eps))
            musq = small.tile([H, 1], F32, tag="musq", name=f"mq{li}")
            nc.vector.tensor_tensor(out=musq[:], in0=mulin[:], in1=mulin[:], op=OP.mult)
            var = small.tile([H, 1], F32, tag="var", name=f"vr{li}")
            nc.vector.tensor_tensor(out=var[:], in0=e2n[:], in1=musq[:], op=OP.subtract)
            nc.vector.tensor_scalar(out=var[:], in0=var[:], scalar1=EPS, scalar2=None, op0=OP.add)
            rec = small.tile([H, 1], F32, tag="rec", name=f"rc{li}")
            nc.vector.reciprocal(out=rec[:], in_=var[:])
            rstd = small.tile([H, 1], F32, tag="rstd", name=f"rs{li}")
            nc.scalar.activation(out=rstd[:], in_=rec[:], func=AF.Sqrt)
            scale = small.tile([H, 1], F32, tag="scale", name=f"sc{li}")
            nc.vector.tensor_tensor(out=scale[:], in0=gcol, in1=rstd[:], op=OP.mult)
            # bias2 = beta - mulin * scale
            t = small.tile([H, 1], F32, tag="tb", name=f"tb{li}")
            nc.vector.tensor_tensor(out=t[:], in0=mulin[:], in1=scale[:], op=OP.mult)
            bias2 = small.tile([H, 1], F32, tag="bias2", name=f"b2{li}")
            nc.vector.tensor_tensor(out=bias2[:], in0=betcol, in1=t[:], op=OP.subtract)
            return scale, bias2

        # ================== the forward pass ==================
        for rep in range(reps):
            # ---- layer 0: encoder + projection, feat-major ----
            CH = 512
            nch = 0 if os.environ.get("GCN_NOENC") else (S + CH - 1) // CH
            for ci in range(nch):
                c0 = ci * CH
                w = min(CH, S - c0)
                ft = stage.tile([IN, CH], F32, tag="ft", name=f"ft{ci}")
                nc.sync.dma_start(out=ft[:, :w], in_=featT_d[:, c0 : c0 + w])
                e1p = pmain.tile([IN // 2, CH], F32, tag="mm", space="PSUM", name=f"e1p{ci}")
                nc.tensor.matmul(out=e1p[:, :w], lhsT=wenc1[:], rhs=ft[:, :w], start=True, stop=True)
                e1 = stage.tile([IN // 2, CH], F32, tag="e1", name=f"e1{ci}")
                nc.scalar.activation(out=e1[:, :w], in_=e1p[:, :w], func=AF.Relu, bias=benc1[:])
                e2p = pmain.tile([IN // 4, CH], F32, tag="mm", space="PSUM", name=f"e2p{ci}")
                nc.tensor.matmul(out=e2p[:, :w], lhsT=wenc2[:], rhs=e1[:, :w], start=True, stop=True)
                e2 = stage.tile([IN // 4, CH], F32, tag="e2", name=f"e2{ci}")
                nc.scalar.activation(out=e2[:, :w], in_=e2p[:, :w], func=AF.Relu, bias=benc2[:])
                h0p = pmain.tile([IN, CH], F32, tag="mm", space="PSUM", name=f"h0p{ci}")
                nc.tensor.matmul(out=h0p[:, :w], lhsT=wenc3[:], rhs=e2[:, :w], start=True, stop=True)
                h0 = stage.tile([IN, CH], F32, tag="h0", name=f"h0{ci}")
                nc.scalar.activation(out=h0[:, :w], in_=h0p[:, :w], func=AF.Identity, bias=benc3[:])
                zp = pmain.tile([H, CH], F32, tag="mm", space="PSUM", name=f"zp{ci}")
                nc.tensor.matmul(out=zp[:, :w], lhsT=w0[:], rhs=h0[:, :w], start=True, stop=True)
                zt = stage.tile([H, CH], F32, tag="zt", name=f"zt{ci}")
                nc.vector.tensor_copy(out=zt[:, :w], in_=zp[:, :w])
                for k in range((w + BLK - 1) // BLK):
                    b = (c0 // BLK) + k
                    bs = bs_of(b)
                    zb_ps = transpose(zt[:, k * BLK : k * BLK + bs], H, bs, f"zb{ci}_{k}")
                    g0 = stage.tile([128, H], F32, tag="g0", name=f"g0{ci}_{k}")
                    nc.vector.tensor_scalar(
                        out=g0[:bs, :],
                        in0=zb_ps[:bs, :],
                        scalar1=outnorm[:bs, b : b + 1],
                        scalar2=None,
                        op0=OP.mult,
                    )
                    nc.sync.dma_start(
                        out=g_local[b * BLK : b * BLK + bs, :], in_=g0[:bs, :]
                    )

            if STAGE < 2:
                continue
            if not os.environ.get("GCN_NOAG"):
                nc.gpsimd.collective_compute(
                    "AllGather", OP.bypass, replica_groups=RG,
                    ins=[g_local[:]], outs=[g_full[:]],
                )

            if STAGE < 3:
                continue
            sum_ps = sumsq_ps = None
            if SPMM_MODE == "full":
                sum_ps = pstat.tile([H, 1], F32, tag="cov", space="PSUM", name=f"s0_{rep}")
                sumsq_ps = pstat.tile([H, 1], F32, tag="sumv", space="PSUM", name=f"q0_{rep}")

            def epi_l0(b, acc):
                bs = bs_of(b)
                asl = aggb[:, b * H : (b + 1) * H]
                nc.vector.tensor_scalar(
                    out=asl, in0=acc[:], scalar1=innorm[:, b : b + 1], scalar2=None, op0=OP.mult
                )
                sq = stage.tile([128, H], F32, tag="sq", name=f"sq{b}")
                nc.scalar.activation(out=sq[:], in_=asl, func=AF.Square)
                nc.tensor.matmul(
                    out=sum_ps[:], lhsT=asl, rhs=ones[:], start=(b == 0), stop=(b == NBLK - 1)
                )
                nc.tensor.matmul(
                    out=sumsq_ps[:], lhsT=sq[:], rhs=ones[:], start=(b == 0), stop=(b == NBLK - 1)
                )

            spmm(g_full, H, epi_l0, rep)
            if STAGE < 4:
                continue
            if SPMM_MODE == "full":
                scale, bias2 = stats_to_scale_bias(0, sum_ps, None, sumsq_ps, rep)
            else:
                scale = small.tile([H, 1], F32, tag="scale", name=f"dsc0_{rep}")
                nc.vector.memset(scale[:], 1.0)
                bias2 = small.tile([H, 1], F32, tag="bias2", name=f"db0_{rep}")
                nc.vector.memset(bias2[:], 0.0)

            for b in range(NBLK):
                bs = bs_of(b)
                aggT_ps = transpose(aggb[:, b * H : (b + 1) * H], 128, H, f"aT0{b}")
                h1T = stage.tile([H, 128], F32, tag="hnT", name=f"h1T{b}")
                nc.scalar.activation(
                    out=h1T[:], in_=aggT_ps[:], func=AF.Relu, scale=scale[:], bias=bias2[:]
                )
                hn_ps = transpose(h1T[:], H, 128, f"hn0{b}")
                build_P(b, hn_ps, False, rep)

            # ---- layers 1..L-1 ----
            if STAGE < 5:
                continue
            for layer in range(1, L):
                last = layer == L - 1
                w_l = wrest[:, (layer - 1) * H : layer * H]
                nc.gpsimd.collective_compute(
                    "AllGather", OP.bypass, replica_groups=RG,
                    ins=[P_local[:]], outs=[P_full[:]],
                )

                def epi_a(b, acc, layer=layer):
                    bs = bs_of(b)
                    hsl = hbuf[:, b * H : (b + 1) * H]
                    comb = stage.tile([128, H2], F32, tag="comb", name=f"cb{layer}_{b}")
                    nc.vector.tensor_scalar(
                        out=comb[:, H:], in0=acc[:, :H],
                        scalar1=rindeg[:, b : b + 1], scalar2=None, op0=OP.mult,
                    )
                    nc.vector.tensor_copy(out=comb[:, :H], in_=hsl)
                    nc.vector.tensor_copy(
                        out=sonh[:, b * H : (b + 1) * H], in_=acc[:, H:]
                    )
                    cT_ps = transpose(comb[:], 128, H2, f"cT{layer}_{b}")
                    cT = stage.tile([H2, 128], F32, tag="combT", name=f"cTs{layer}_{b}")
                    nc.vector.tensor_copy(out=cT[:], in_=cT_ps[:])
                    a1p = pmain.tile([H, 128], F32, tag="mm", space="PSUM", name=f"a1p{layer}_{b}")
                    nc.tensor.matmul(out=a1p[:], lhsT=watt1[:], rhs=cT[:], start=True, stop=True)
                    a1 = stage.tile([H, 128], F32, tag="a1", name=f"a1{layer}_{b}")
                    nc.scalar.activation(out=a1[:], in_=a1p[:], func=AF.Relu, bias=batt1[:])
                    a2p = pmain.tile([1, 128], F32, tag="mm", space="PSUM", name=f"a2p{layer}_{b}")
                    nc.tensor.matmul(out=a2p[:], lhsT=watt2[:], rhs=a1[:], start=True, stop=True)
                    a2 = stage.tile([1, 128], F32, tag="a2", name=f"a2{layer}_{b}")
                    nc.scalar.activation(out=a2[:], in_=a2p[:], func=AF.Sigmoid, bias=batt2[:])
                    aNp = pmain.tile([128, 1], F32, tag="mm", space="PSUM", name=f"aNp{layer}_{b}")
                    nc.tensor.matmul(out=aNp[:], lhsT=a2[:], rhs=ident[:1, :1], start=True, stop=True)
                    aN = stage.tile([128, 1], F32, tag="aN", name=f"aN{layer}_{b}")
                    nc.vector.tensor_copy(out=aN[:], in_=aNp[:])
                    anb = stage.tile([128, H], F32, tag="anb", name=f"anb{layer}_{b}")
                    nc.vector.tensor_scalar(
                        out=anb[:], in0=comb[:, H:], scalar1=aN[:], scalar2=None, op0=OP.mult
                    )
                    nc.vector.tensor_tensor(out=hsl, in0=hsl, in1=anb[:], op=OP.add)
                    g2 = stage.tile([128, H], F32, tag="g0", name=f"g2{layer}_{b}")
                    nc.vector.tensor_scalar(
                        out=g2[:], in0=anb[:], scalar1=outnorm[:, b : b + 1], scalar2=None, op0=OP.mult
                    )
                    nc.sync.dma_start(
                        out=g_local[b * BLK : b * BLK + bs, :], in_=g2[:bs, :]
                    )

                spmm(P_full, H2, epi_a, rep)
                if STAGE < 6:
                    break
                nc.gpsimd.collective_compute(
                    "AllGather", OP.bypass, replica_groups=RG,
                    ins=[g_local[:]], outs=[g_full[:]],
                )

                cov_ps = sumv_ps = None
                if SPMM_MODE == "full":
                    cov_ps = pstat.tile([H, H], F32, tag="cov", space="PSUM", name=f"cv{layer}_{rep}")
                    sumv_ps = pstat.tile([H, 1], F32, tag="sumv", space="PSUM", name=f"sv{layer}_{rep}")

                def epi_b(b, acc, layer=layer):
                    asl = aggb[:, b * H : (b + 1) * H]
                    tt = stage.tile([128, H], F32, tag="sq", name=f"tt{layer}_{b}")
                    nc.vector.tensor_tensor(
                        out=tt[:], in0=acc[:], in1=sonh[:, b * H : (b + 1) * H], op=OP.add
                    )
                    nc.vector.tensor_scalar(
                        out=asl, in0=tt[:], scalar1=innorm[:, b : b + 1], scalar2=None, op0=OP.mult
                    )
                    nc.tensor.matmul(
                        out=cov_ps[:], lhsT=asl, rhs=asl, start=(b == 0), stop=(b == NBLK - 1)
                    )
                    nc.tensor.matmul(
                        out=sumv_ps[:], lhsT=asl, rhs=ones[:], start=(b == 0), stop=(b == NBLK - 1)
                    )

                spmm(g_full, H, epi_b, rep)
                if STAGE < 7:
                    break
                if SPMM_MODE == "full":
                    scale, bias2 = stats_to_scale_bias(layer, sumv_ps, cov_ps, None, rep)
                else:
                    scale = small.tile([H, 1], F32, tag="scale", name=f"dsc{layer}_{rep}")
                    nc.vector.memset(scale[:], 1.0)
                    bias2 = small.tile([H, 1], F32, tag="bias2", name=f"db{layer}_{rep}")
                    nc.vector.memset(bias2[:], 0.0)

                for b in range(NBLK):
                    bs = bs_of(b)
                    aggT_ps = transpose(aggb[:, b * H : (b + 1) * H], 128, H, f"aT{layer}_{b}")
                    aggT = stage.tile([H, 128], F32, tag="aggT", name=f"aTs{layer}_{b}")
                    nc.vector.tensor_copy(out=aggT[:], in_=aggT_ps[:])
                    linp = pmain.tile([H, 128], F32, tag="mm", space="PSUM", name=f"lp{layer}_{b}")
                    nc.tensor.matmul(out=linp[:], lhsT=w_l, rhs=aggT[:], start=True, stop=True)
                    t2 = stage.tile([H, 128], F32, tag="t2", name=f"t2{layer}_{b}")
                    nc.vector.tensor_scalar(
                        out=t2[:], in0=linp[:], scalar1=scale[:], scalar2=bias2[:],
                        op0=OP.mult, op1=OP.add,
                    )
                    hpT_ps = transpose(hbuf[:, b * H : (b + 1) * H], 128, H, f"hpT{layer}_{b}")
                    t3 = stage.tile([H, 128], F32, tag="t3", name=f"t3{layer}_{b}")
                    nc.vector.tensor_tensor(out=t3[:], in0=t2[:], in1=hpT_ps[:], op=OP.add)
                    hnT = stage.tile([H, 128], F32, tag="hnT", name=f"hnT{layer}_{b}")
                    nc.scalar.activation(out=hnT[:], in_=t3[:], func=AF.Relu)
                    if last:
                        op_ = pmain.tile([C, 128], F32, tag="mm", space="PSUM", name=f"op{layer}_{b}")
                        nc.tensor.matmul(out=op_[:], lhsT=wfc[:], rhs=hnT[:], start=True, stop=True)
                        ot = stage.tile([C, 128], F32, tag="ot", name=f"ot{layer}_{b}")
                        nc.scalar.activation(out=ot[:], in_=op_[:], func=AF.Identity, bias=bfc[:])
                        nc.sync.dma_start(
                            out=outT_d[:, b * BLK : b * BLK + bs], in_=ot[:, :bs]
                        )
                    else:
                        hn_ps = transpose(hnT[:], H, 128, f"hn{layer}_{b}")
                        build_P(b, hn_ps, False, rep)

        ctx.close()

    return nc


# --------------------------------------------------------------------------
def _make_in_maps(meta, inputs, IN, H, C, L):
    N, S = meta["N"], meta["S"]
    f = lambda x: np.ascontiguousarray(np.asarray(x, dtype=np.float32))
    feats = f(inputs["features"])
    W_rest = f(inputs["W_rest"])
    b_rest = f(inputs["b_rest"])
    bvec = np.concatenate([f(inputs["b0"])[None, :], b_rest], axis=0).T.copy()  # [H, L]
    iota = np.tile(np.arange(128, dtype=np.float32)[None, :], (128, 1))
    ident = np.eye(128, dtype=np.float32)
    ones = np.ones((128, 1), np.float32)
    shared = dict(
        iota=iota,
        ident=ident,
        ones=ones,
        wenc1=f(inputs["enc_W1"]),
        benc1=f(inputs["enc_b1"])[:, None],
        wenc2=f(inputs["enc_W2"]),
        benc2=f(inputs["enc_b2"])[:, None],
        wenc3=f(inputs["enc_W3"]),
        benc3=f(inputs["enc_b3"])[:, None],
        w0=f(inputs["W0"]),
        bvec=bvec,
        wrest=np.ascontiguousarray(W_rest.transpose(1, 0, 2).reshape(W_rest.shape[1], -1)),
        gam=np.ascontiguousarray(f(inputs["gamma"]).T),
        bet=np.ascontiguousarray(f(inputs["beta"]).T),
        watt1=f(inputs["att_W1"]),
        batt1=f(inputs["att_b1"])[:, None],
        watt2=f(inputs["att_W2"]),
        batt2=f(inputs["att_b2"])[:, None],
        wfc=f(inputs["fc_W"]),
        bfc=f(inputs["fc_b"])[:, None],
    )
    in_maps = []
    for c in range(NCORES):
        m = dict(shared)
        m["featT"] = np.ascontiguousarray(feats[c * S : (c + 1) * S].T)
        m["idx"] = np.ascontiguousarray(meta["idx"][c])
        m["seg"] = np.ascontiguousarray(meta["seg"][c])
        m["innorm"] = np.ascontiguousarray(meta["in_norm_b"][c])
        m["outnorm"] = np.ascontiguousarray(meta["out_norm_b"][c])
        m["rindeg"] = np.ascontiguousarray(meta["r_indeg_b"][c])
        in_maps.append(m)
    return in_maps


def _prep_all(inputs, reps=1):
    feats = np.asarray(inputs["features"])
    N, IN = feats.shape
    H = np.asarray(inputs["W0"]).shape[1]
    C = np.asarray(inputs["fc_W"]).shape[1]
    L = np.asarray(inputs["gamma"]).shape[0]
    src = np.asarray(inputs["src"]).astype(np.int64)
    dst = np.asarray(inputs["dst"]).astype(np.int64)
    meta = _preprocess(N, src, dst)
    nc = _build_program(meta, IN, H, C, L, reps=reps)
    nc.compile()
    _split_excess_waits(nc)
    in_maps = _make_in_maps(meta, inputs, IN, H, C, L)
    return meta, nc, in_maps, (IN, H, C, L)


def kernel(**inputs):
    meta, nc, in_maps, (IN, H, C, L) = _prep_all(inputs, reps=1)
    res = run_bass_kernel_spmd(nc, in_maps, list(range(NCORES)))
    S, N = meta["S"], meta["N"]
    out = np.empty((N, C), np.float32)
    for c in range(NCORES):
        out[c * S : (c + 1) * S] = res.results[c]["outT"].T
    return out



# revision 1
# speedup vs baseline: 1.3824x; 1.3824x over previous
"""Distributed EnhancedResGCN forward for 8 Trainium2 NeuronCores.

Strategy (graph/data parallel over nodes, per the sharding hint):
  - nodes are sharded contiguously across 8 cores; all dense per-node math is
    done on the owning core with replicated weights.
  - each SpMM (7 total: 1 conv for layer 0 via the projection trick, then
    neighbor-mean + conv-delta for layers 1..3) is done dst-side: the needed
    per-node feature tables are AllGathered (node rows, [N,64] or [N,128]),
    then each core gathers rows for its own edges with dma_gather and reduces
    them into per-128-node-block PSUM accumulators with one-hot matmuls
    (edges sorted by destination block on the host).
  - BatchNorm statistics are computed from per-shard second-moment matmuls and
    combined with a tiny AllReduce.

Host-side preprocessing (numpy) builds the edge tiling: edges grouped by
(dst block, src bucket of 25000 so gather indices fit int16), padded to
128-edge tiles with a structure that is identical across all 8 cores (SPMD:
one program, per-core data).
"""

import os
import sys

for _p in ("/opt/trn_rl_repo", "/root/.axon_site/_ro/trn_rl_repo"):
    if os.path.isdir(_p) and _p not in sys.path:
        sys.path.append(_p)

import numpy as np

import concourse.bass as bass
import concourse.tile as tile
from concourse import bacc, mybir
from concourse.bass_utils import run_bass_kernel_spmd

try:
    import bass_rust
except ImportError:  # pragma: no cover
    bass_rust = None

F32 = mybir.dt.float32
I16 = mybir.dt.int16
AF = mybir.ActivationFunctionType
OP = mybir.AluOpType

NCORES = 8
BLK = 128
BUCKET = 25000
MAX_PHASE_TILES = 36
EPS = 1e-5
STAGE = int(os.environ.get("GCN_STAGE", "99"))
SPMM_MODE = os.environ.get("GCN_SPMM", "full")  # gather | mm | full


# --------------------------------------------------------------------------
# walrus in this container rejects instructions carrying >1 sem wait; hoist
# extras onto same-engine NOPs inserted right before the instruction.
def _split_excess_waits(nc, max_waits=1):
    n_split = 0
    for fn in nc.m.functions:
        for blk in fn.blocks:
            insts = blk.instructions
            pos = 0
            while pos < len(insts):
                inst = insts[pos]
                si = inst.sync_info
                waits = list(si.on_wait) if si is not None and si.on_wait else []
                if len(waits) > max_waits:
                    si.on_wait = waits[:max_waits]
                    extra = waits[max_waits:]
                    at = pos
                    for j in range(0, len(extra), max_waits):
                        nop = mybir.InstNoOp(
                            name=f"waitnop_{n_split}_{j}", ins=[], outs=[]
                        )
                        nop.engine = inst.engine
                        nop.sync_info = bass_rust.SyncInfo(
                            on_wait=extra[j : j + max_waits], on_update=[]
                        )
                        try:
                            nc.register_instruction(nop, overwrite=True)
                        except Exception:
                            pass
                        insts.insert(at, nop)
                        at += 1
                        pos += 1
                    n_split += 1
                pos += 1
    return n_split


# --------------------------------------------------------------------------
# host-side edge preprocessing
def _preprocess(N, src, dst):
    S = N // NCORES
    NBLK = (S + BLK - 1) // BLK
    nbuck = (N + BUCKET - 1) // BUCKET
    bucket_sizes = [min(BUCKET, N - j * BUCKET) for j in range(nbuck)]

    in_deg = np.bincount(dst, minlength=N).astype(np.float64)
    out_deg = np.bincount(src, minlength=N).astype(np.float64)
    in_deg_c = np.maximum(in_deg, 1.0)
    out_deg_c = np.maximum(out_deg, 1.0)
    in_norm = (in_deg_c**-0.5).astype(np.float32)
    out_norm = (out_deg_c**-0.5).astype(np.float32)
    r_indeg = (1.0 / in_deg_c).astype(np.float32)

    # per-core sorted edges + group boundaries on the (block, bucket) key
    percore = []
    counts = np.zeros((NCORES, NBLK * nbuck), np.int64)
    for c in range(NCORES):
        m = (dst >= c * S) & (dst < (c + 1) * S)
        es = src[m]
        ed = dst[m] - c * S
        bl = ed // BLK
        bu = es // BUCKET
        key = bl * nbuck + bu
        order = np.lexsort((es, key))
        es, ed, key = es[order], ed[order], key[order]
        starts = np.searchsorted(key, np.arange(NBLK * nbuck))
        ends = np.searchsorted(key, np.arange(NBLK * nbuck) + 1)
        counts[c] = ends - starts
        percore.append((es, ed, starts, ends))

    maxc = counts.max(axis=0).reshape(NBLK, nbuck)
    tiles_per = (maxc + BLK - 1) // BLK  # [NBLK, nbuck]
    # ensure every block has at least one tile so PSUM accum groups exist
    empty = tiles_per.sum(axis=1) == 0
    tiles_per[empty, 0] = 1

    # phase packing: whole blocks, limited total tiles
    blk_tiles = tiles_per.sum(axis=1)
    phases = []
    cur, cur_t = [], 0
    for b in range(NBLK):
        if cur and cur_t + blk_tiles[b] > MAX_PHASE_TILES:
            phases.append(cur)
            cur, cur_t = [], 0
        cur.append(b)
        cur_t += blk_tiles[b]
    if cur:
        phases.append(cur)

    # global tile order: phase -> bucket -> block
    phase_meta = []
    T = 0
    for blist in phases:
        calls = []
        pcol = 0
        block_tiles = {b: [] for b in blist}
        for j in range(nbuck):
            cnt = int(sum(tiles_per[b, j] for b in blist))
            if cnt == 0:
                continue
            calls.append(dict(bucket=j, off=pcol, cnt=cnt, gtile=T))
            for b in blist:
                for _ in range(int(tiles_per[b, j])):
                    block_tiles[b].append((pcol, T))
                    pcol += 1
                    T += 1
        phase_meta.append(
            dict(
                blocks=[dict(b=b, tiles=block_tiles[b]) for b in blist],
                ntiles=pcol,
                calls=calls,
            )
        )

    # per-core idx/seg arrays in the same global order
    idx_all = np.zeros((NCORES, T, 128), np.int16)
    seg_all = np.full((NCORES, T, 128), 255.0, np.float32)
    for c in range(NCORES):
        es, ed, starts, ends = percore[c]
        t_cursor = 0
        for ph in phase_meta:
            for call in ph["calls"]:
                j = call["bucket"]
                for binfo in ph["blocks"]:
                    b = binfo["b"]
                    nt = int(tiles_per[b, j])
                    if nt == 0:
                        continue
                    g = b * nbuck + j
                    s, e = int(starts[g]), int(ends[g])
                    cnt = e - s
                    loc_idx = (es[s:e] - j * BUCKET).astype(np.int16)
                    loc_seg = (ed[s:e] - b * BLK).astype(np.float32)
                    pad_idx = loc_idx[-1] if cnt > 0 else np.int16(0)
                    block_idx = np.full(nt * 128, pad_idx, np.int16)
                    block_seg = np.full(nt * 128, 255.0, np.float32)
                    block_idx[:cnt] = loc_idx
                    block_seg[:cnt] = loc_seg
                    idx_all[c, t_cursor : t_cursor + nt] = block_idx.reshape(nt, 128)
                    seg_all[c, t_cursor : t_cursor + nt] = block_seg.reshape(nt, 128)
                    t_cursor += nt
        assert t_cursor == T

    # wrap idx: per tile [128] -> [16, 8] (pos i -> [i%16, i//16]), concat along
    # cols, replicate to 128 partitions.
    idx_wrapped = np.zeros((NCORES, 128, T * 8), np.int16)
    w = idx_all.reshape(NCORES, T, 8, 16).transpose(0, 3, 1, 2).reshape(NCORES, 16, T * 8)
    for r in range(8):
        idx_wrapped[:, 16 * r : 16 * (r + 1), :] = w
    seg_cols = seg_all.transpose(0, 2, 1).copy()  # [NCORES, 128, T]

    def block_major(vec):  # [S] -> [128, NBLK], zero-padded
        out = np.zeros((NCORES, 128, NBLK), np.float32)
        for c in range(NCORES):
            v = vec[c * S : (c + 1) * S]
            pad = np.zeros(NBLK * BLK, np.float32)
            pad[:S] = v
            out[c] = pad.reshape(NBLK, BLK).T
        return out

    return dict(
        N=N,
        S=S,
        NBLK=NBLK,
        nbuck=nbuck,
        bucket_sizes=bucket_sizes,
        phases=phase_meta,
        T=T,
        idx=idx_wrapped,
        seg=seg_cols,
        in_norm_b=block_major(in_norm),
        out_norm_b=block_major(out_norm),
        r_indeg_b=block_major(r_indeg),
    )


# --------------------------------------------------------------------------
def _build_program(meta, IN, H, C, L, reps=1):
    N, S, NBLK = meta["N"], meta["S"], meta["NBLK"]
    T = meta["T"]
    H2 = 2 * H
    n_inv = 1.0 / N

    nc = bacc.Bacc("TRN2", target_bir_lowering=False, debug=False, num_devices=NCORES)

    # ---- I/O ----
    featT_d = nc.dram_tensor("featT", [IN, S], F32, kind="ExternalInput")
    idx_d = nc.dram_tensor("idx", [128, T * 8], I16, kind="ExternalInput")
    seg_d = nc.dram_tensor("seg", [128, T], F32, kind="ExternalInput")
    innorm_d = nc.dram_tensor("innorm", [128, NBLK], F32, kind="ExternalInput")
    outnorm_d = nc.dram_tensor("outnorm", [128, NBLK], F32, kind="ExternalInput")
    rindeg_d = nc.dram_tensor("rindeg", [128, NBLK], F32, kind="ExternalInput")
    iota_d = nc.dram_tensor("iota", [128, 128], F32, kind="ExternalInput")
    ident_d = nc.dram_tensor("ident", [128, 128], F32, kind="ExternalInput")
    ones_d = nc.dram_tensor("ones", [128, 1], F32, kind="ExternalInput")
    wenc1_d = nc.dram_tensor("wenc1", [IN, IN // 2], F32, kind="ExternalInput")
    benc1_d = nc.dram_tensor("benc1", [IN // 2, 1], F32, kind="ExternalInput")
    wenc2_d = nc.dram_tensor("wenc2", [IN // 2, IN // 4], F32, kind="ExternalInput")
    benc2_d = nc.dram_tensor("benc2", [IN // 4, 1], F32, kind="ExternalInput")
    wenc3_d = nc.dram_tensor("wenc3", [IN // 4, IN], F32, kind="ExternalInput")
    benc3_d = nc.dram_tensor("benc3", [IN, 1], F32, kind="ExternalInput")
    w0_d = nc.dram_tensor("w0", [IN, H], F32, kind="ExternalInput")
    b_d = nc.dram_tensor("bvec", [H, L], F32, kind="ExternalInput")  # unused cols kept for uniform I/O
    wr_d = nc.dram_tensor("wrest", [H, (L - 1) * H], F32, kind="ExternalInput")
    gam_d = nc.dram_tensor("gam", [H, L], F32, kind="ExternalInput")
    bet_d = nc.dram_tensor("bet", [H, L], F32, kind="ExternalInput")
    watt1_d = nc.dram_tensor("watt1", [H2, H], F32, kind="ExternalInput")
    batt1_d = nc.dram_tensor("batt1", [H, 1], F32, kind="ExternalInput")
    watt2_d = nc.dram_tensor("watt2", [H, 1], F32, kind="ExternalInput")
    batt2_d = nc.dram_tensor("batt2", [1, 1], F32, kind="ExternalInput")
    wfc_d = nc.dram_tensor("wfc", [H, C], F32, kind="ExternalInput")
    bfc_d = nc.dram_tensor("bfc", [C, 1], F32, kind="ExternalInput")
    outT_d = nc.dram_tensor("outT", [C, S], F32, kind="ExternalOutput")

    # internal DRAM
    P_local = nc.dram_tensor("P_local", [S, H2], F32)
    P_full = nc.dram_tensor("P_full", [N, H2], F32, addr_space=os.environ.get("GCN_AS", "Shared"))
    g_local = nc.dram_tensor("g_local", [S, H], F32)
    g_full = nc.dram_tensor("g_full", [N, H], F32, addr_space=os.environ.get("GCN_AS", "Shared"))
    st_local = nc.dram_tensor("st_local", [H, H + 2], F32)
    st_full = nc.dram_tensor("st_full", [H, H + 2], F32, addr_space=os.environ.get("GCN_AS", "Shared"))

    RG = [list(range(NCORES))]

    with tile.TileContext(nc) as tc:
        import contextlib

        ctx = contextlib.ExitStack()
        const = ctx.enter_context(tc.tile_pool(name="const", bufs=1))
        persist = ctx.enter_context(tc.tile_pool(name="persist", bufs=1))
        gath = ctx.enter_context(tc.tile_pool(name="gath", bufs=2))
        mpool = ctx.enter_context(tc.tile_pool(name="mpool", bufs=6))
        stage = ctx.enter_context(tc.tile_pool(name="stage", bufs=3))
        small = ctx.enter_context(tc.tile_pool(name="small", bufs=2))
        pmain = ctx.enter_context(tc.tile_pool(name="pmain", bufs=2, space="PSUM"))
        pstat = ctx.enter_context(tc.tile_pool(name="pstat", bufs=1, space="PSUM"))

        def load_const(dram, shape, dtype=F32, name=None):
            t = const.tile(shape, dtype, name=name or dram.name + "_s")
            nc.sync.dma_start(out=t[:], in_=dram[:])
            return t

        idx_s = load_const(idx_d, [128, T * 8], I16)
        seg_s = load_const(seg_d, [128, T])
        innorm = load_const(innorm_d, [128, NBLK])
        outnorm = load_const(outnorm_d, [128, NBLK])
        rindeg = load_const(rindeg_d, [128, NBLK])
        iota = load_const(iota_d, [128, 128])
        ident = load_const(ident_d, [128, 128])
        ones = load_const(ones_d, [128, 1])
        wenc1 = load_const(wenc1_d, [IN, IN // 2])
        benc1 = load_const(benc1_d, [IN // 2, 1])
        wenc2 = load_const(wenc2_d, [IN // 2, IN // 4])
        benc2 = load_const(benc2_d, [IN // 4, 1])
        wenc3 = load_const(wenc3_d, [IN // 4, IN])
        benc3 = load_const(benc3_d, [IN, 1])
        w0 = load_const(w0_d, [IN, H])
        bvec = load_const(b_d, [H, L])
        wrest = load_const(wr_d, [H, (L - 1) * H])
        gam = load_const(gam_d, [H, L])
        bet = load_const(bet_d, [H, L])
        watt1 = load_const(watt1_d, [H2, H])
        batt1 = load_const(batt1_d, [H, 1])
        watt2 = load_const(watt2_d, [H, 1])
        batt2 = load_const(batt2_d, [1, 1])
        wfc = load_const(wfc_d, [H, C])
        bfc = load_const(bfc_d, [C, 1])

        hbuf = persist.tile([128, NBLK * H], F32, name="hbuf")
        sonh = persist.tile([128, NBLK * H], F32, name="sonh")
        aggb = persist.tile([128, NBLK * H], F32, name="aggb")
        if SPMM_MODE != "full":
            nc.vector.memset(hbuf[:], 0.0)
            nc.vector.memset(sonh[:], 0.0)
            nc.vector.memset(aggb[:], 0.0)

        def bs_of(b):  # valid rows in block b
            return min(BLK, S - b * BLK)

        def spmm(table_ap, elem, epilogue, rep):
            """gather+scatter over all edges; epilogue(b, acc_psum) per block."""
            for pi, ph in enumerate(meta["phases"]):
                g = gath.tile([128, ph["ntiles"], elem], F32, tag="gath", name=f"g_{rep}_{elem}_{pi}")
                for call in ph["calls"]:
                    j = call["bucket"]
                    bsz = meta["bucket_sizes"][j]
                    off, cnt, gt = call["off"], call["cnt"], call["gtile"]
                    nc.gpsimd.dma_gather(
                        g[:, off : off + cnt, :],
                        table_ap[j * BUCKET : j * BUCKET + bsz, :],
                        idx_s[:, gt * 8 : (gt + cnt) * 8],
                        cnt * 128,
                        cnt * 128,
                        elem,
                        single_packet=False,
                    )
                if SPMM_MODE == "gather":
                    continue
                for binfo in ph["blocks"]:
                    b = binfo["b"]
                    acc = pmain.tile([128, elem], F32, tag="accum", name=f"acc{b}")
                    ntl = len(binfo["tiles"])
                    for k, (pcol, gt) in enumerate(binfo["tiles"]):
                        m = mpool.tile([128, 128], F32, tag="m", name=f"m{b}_{k}")
                        nc.vector.tensor_tensor(
                            out=m[:],
                            in0=iota[:],
                            in1=seg_s[:, gt : gt + 1].to_broadcast([128, 128]),
                            op=OP.is_equal,
                        )
                        nc.tensor.matmul(
                            out=acc[:],
                            lhsT=m[:],
                            rhs=g[:, pcol, :],
                            start=(k == 0),
                            stop=(k == ntl - 1),
                        )
                    if SPMM_MODE != "mm":
                        epilogue(b, acc)

        def transpose(src_ap, p, f, name):
            """src [p, f] (SBUF) -> psum tile [f, p]"""
            tp = pmain.tile([f, p], F32, tag="trans", space="PSUM", name=name)
            nc.tensor.transpose(out=tp[:], in_=src_ap, identity=ident[:p, :p])
            return tp

        def build_P(b, hn_ps, last_layer, rep):
            """write h_next (psum [128,H]) into hbuf + P_local rows"""
            bs = bs_of(b)
            hsl = hbuf[:, b * H : (b + 1) * H]
            nc.vector.tensor_copy(out=hsl, in_=hn_ps[:])
            if last_layer:
                return
            pst = stage.tile([128, H2], F32, tag="pst", name=f"pst{b}")
            nc.vector.tensor_copy(out=pst[:, :H], in_=hn_ps[:])
            nc.vector.tensor_scalar(
                out=pst[:, H:],
                in0=hn_ps[:],
                scalar1=outnorm[:, b : b + 1],
                scalar2=None,
                op0=OP.mult,
            )
            nc.sync.dma_start(
                out=P_local[b * BLK : b * BLK + bs, :], in_=pst[:bs, :]
            )

        def stats_to_scale_bias(layer, sum_ps, cov_ps, sumsq_ps, rep):
            """AllReduce moments -> scale [H,1], bias2 [H,1] (SBUF)."""
            li = str(layer) + "_" + str(rep)
            st_s = stage.tile([H, H + 2], F32, tag="stats", name=f"st{li}")
            if cov_ps is not None:
                nc.vector.tensor_copy(out=st_s[:, :H], in_=cov_ps[:])
            else:
                nc.vector.memset(st_s[:, :H], 0.0)
            nc.vector.tensor_copy(out=st_s[:, H : H + 1], in_=sum_ps[:])
            if sumsq_ps is not None:
                nc.vector.tensor_copy(out=st_s[:, H + 1 : H + 2], in_=sumsq_ps[:])
            else:
                nc.vector.memset(st_s[:, H + 1 : H + 2], 0.0)
            nc.sync.dma_start(out=st_local[:], in_=st_s[:])
            nc.gpsimd.collective_compute(
                "AllReduce",
                OP.add,
                replica_groups=RG,
                ins=[st_local[:]],
                outs=[st_full[:]],
            )
            stg = stage.tile([H, H + 2], F32, tag="stats2", name=f"stg{li}")
            nc.sync.dma_start(out=stg[:], in_=st_full[:])

            # gamma/beta are stored column-major [H, L]; the conv bias b
            # cancels inside BatchNorm and is not needed.
            gcol = gam[:, layer : layer + 1]
            betcol = bet[:, layer : layer + 1]

            mulin = small.tile([H, 1], F32, tag="mulin", name=f"ml{li}")
            e2n = small.tile([H, 1], F32, tag="e2n", name=f"e2{li}")
            if cov_ps is not None:
                # mulin = W.T @ (sum/N)
                msum = small.tile([H, 1], F32, tag="msum", name=f"ms{li}")
                nc.vector.tensor_scalar(
                    out=msum[:], in0=stg[:, H : H + 1], scalar1=n_inv, scalar2=None, op0=OP.mult
                )
                w_l = wrest[:, (layer - 1) * H : layer * H]
                ml_ps = pmain.tile([H, 1], F32, tag="mm", space="PSUM", name=f"mlp{li}")
                nc.tensor.matmul(out=ml_ps[:], lhsT=w_l, rhs=msum[:], start=True, stop=True)
                nc.vector.tensor_copy(out=mulin[:], in_=ml_ps[:])
                # E2 = diag(W.T Sigma W)/N
                a_ps = pmain.tile([H, H], F32, tag="mm", space="PSUM", name=f"ap{li}")
                nc.tensor.matmul(out=a_ps[:], lhsT=stg[:, :H], rhs=w_l, start=True, stop=True)
                bmat = stage.tile([H, H], F32, tag="bmat", name=f"bm{li}")
                nc.vector.tensor_tensor(out=bmat[:], in0=a_ps[:], in1=w_l, op=OP.mult)
                e2_ps = pmain.tile([H, 1], F32, tag="mm", space="PSUM", name=f"e2p{li}")
                nc.tensor.matmul(out=e2_ps[:], lhsT=bmat[:], rhs=ones[:H, :], start=True, stop=True)
                nc.vector.tensor_scalar(
                    out=e2n[:], in0=e2_ps[:], scalar1=n_inv, scalar2=None, op0=OP.mult
                )
            else:
                nc.vector.tensor_scalar(
                    out=mulin[:], in0=stg[:, H : H + 1], scalar1=n_inv, scalar2=None, op0=OP.mult
                )
                nc.vector.tensor_scalar(
                    out=e2n[:], in0=stg[:, H + 1 : H + 2], scalar1=n_inv, scalar2=None, op0=OP.mult
                )
            # var = e2n - mulin^2 ; rstd = sqrt(1/(var+eps))
            musq = small.tile([H, 1], F32, tag="musq", name=f"mq{li}")
            nc.vector.tensor_tensor(out=musq[:], in0=mulin[:], in1=mulin[:], op=OP.mult)
            var = small.tile([H, 1], F32, tag="var", name=f"vr{li}")
            nc.vector.tensor_tensor(out=var[:], in0=e2n[:], in1=musq[:], op=OP.subtract)
            nc.vector.tensor_scalar(out=var[:], in0=var[:], scalar1=EPS, scalar2=None, op0=OP.add)
            rec = small.tile([H, 1], F32, tag="rec", name=f"rc{li}")
            nc.vector.reciprocal(out=rec[:], in_=var[:])
            rstd = small.tile([H, 1], F32, tag="rstd", name=f"rs{li}")
            nc.scalar.activation(out=rstd[:], in_=rec[:], func=AF.Sqrt)
            scale = small.tile([H, 1], F32, tag="scale", name=f"sc{li}")
            nc.vector.tensor_tensor(out=scale[:], in0=gcol, in1=rstd[:], op=OP.mult)
            # bias2 = beta - mulin * scale
            t = small.tile([H, 1], F32, tag="tb", name=f"tb{li}")
            nc.vector.tensor_tensor(out=t[:], in0=mulin[:], in1=scale[:], op=OP.mult)
            bias2 = small.tile([H, 1], F32, tag="bias2", name=f"b2{li}")
            nc.vector.tensor_tensor(out=bias2[:], in0=betcol, in1=t[:], op=OP.subtract)
            return scale, bias2

        # ================== the forward pass ==================
        for rep in range(reps):
            # ---- layer 0: encoder + projection, feat-major ----
            CH = 512
            nch = 0 if os.environ.get("GCN_NOENC") else (S + CH - 1) // CH
            for ci in range(nch):
                c0 = ci * CH
                w = min(CH, S - c0)
                ft = stage.tile([IN, CH], F32, tag="ft", name=f"ft{ci}")
                nc.sync.dma_start(out=ft[:, :w], in_=featT_d[:, c0 : c0 + w])
                e1p = pmain.tile([IN // 2, CH], F32, tag="mm", space="PSUM", name=f"e1p{ci}")
                nc.tensor.matmul(out=e1p[:, :w], lhsT=wenc1[:], rhs=ft[:, :w], start=True, stop=True)
                e1 = stage.tile([IN // 2, CH], F32, tag="e1", name=f"e1{ci}")
                nc.scalar.activation(out=e1[:, :w], in_=e1p[:, :w], func=AF.Relu, bias=benc1[:])
                e2p = pmain.tile([IN // 4, CH], F32, tag="mm", space="PSUM", name=f"e2p{ci}")
                nc.tensor.matmul(out=e2p[:, :w], lhsT=wenc2[:], rhs=e1[:, :w], start=True, stop=True)
                e2 = stage.tile([IN // 4, CH], F32, tag="e2", name=f"e2{ci}")
                nc.scalar.activation(out=e2[:, :w], in_=e2p[:, :w], func=AF.Relu, bias=benc2[:])
                h0p = pmain.tile([IN, CH], F32, tag="mm", space="PSUM", name=f"h0p{ci}")
                nc.tensor.matmul(out=h0p[:, :w], lhsT=wenc3[:], rhs=e2[:, :w], start=True, stop=True)
                h0 = stage.tile([IN, CH], F32, tag="h0", name=f"h0{ci}")
                nc.scalar.activation(out=h0[:, :w], in_=h0p[:, :w], func=AF.Identity, bias=benc3[:])
                zp = pmain.tile([H, CH], F32, tag="mm", space="PSUM", name=f"zp{ci}")
                nc.tensor.matmul(out=zp[:, :w], lhsT=w0[:], rhs=h0[:, :w], start=True, stop=True)
                zt = stage.tile([H, CH], F32, tag="zt", name=f"zt{ci}")
                nc.vector.tensor_copy(out=zt[:, :w], in_=zp[:, :w])
                for k in range((w + BLK - 1) // BLK):
                    b = (c0 // BLK) + k
                    bs = bs_of(b)
                    zb_ps = transpose(zt[:, k * BLK : k * BLK + bs], H, bs, f"zb{ci}_{k}")
                    g0 = stage.tile([128, H], F32, tag="g0", name=f"g0{ci}_{k}")
                    nc.vector.tensor_scalar(
                        out=g0[:bs, :],
                        in0=zb_ps[:bs, :],
                        scalar1=outnorm[:bs, b : b + 1],
                        scalar2=None,
                        op0=OP.mult,
                    )
                    nc.sync.dma_start(
                        out=g_local[b * BLK : b * BLK + bs, :], in_=g0[:bs, :]
                    )

            if STAGE < 2:
                continue
            if not os.environ.get("GCN_NOAG"):
                nc.gpsimd.collective_compute(
                    "AllGather", OP.bypass, replica_groups=RG,
                    ins=[g_local[:]], outs=[g_full[:]],
                )

            if STAGE < 3:
                continue
            sum_ps = sumsq_ps = None
            if SPMM_MODE == "full":
                sum_ps = pstat.tile([H, 1], F32, tag="cov", space="PSUM", name=f"s0_{rep}")
                sumsq_ps = pstat.tile([H, 1], F32, tag="sumv", space="PSUM", name=f"q0_{rep}")

            def epi_l0(b, acc):
                bs = bs_of(b)
                asl = aggb[:, b * H : (b + 1) * H]
                nc.vector.tensor_scalar(
                    out=asl, in0=acc[:], scalar1=innorm[:, b : b + 1], scalar2=None, op0=OP.mult
                )
                sq = stage.tile([128, H], F32, tag="sq", name=f"sq{b}")
                nc.scalar.activation(out=sq[:], in_=asl, func=AF.Square)
                nc.tensor.matmul(
                    out=sum_ps[:], lhsT=asl, rhs=ones[:], start=(b == 0), stop=(b == NBLK - 1)
                )
                nc.tensor.matmul(
                    out=sumsq_ps[:], lhsT=sq[:], rhs=ones[:], start=(b == 0), stop=(b == NBLK - 1)
                )

            spmm(g_full, H, epi_l0, rep)
            if STAGE < 4:
                continue
            if SPMM_MODE == "full":
                scale, bias2 = stats_to_scale_bias(0, sum_ps, None, sumsq_ps, rep)
            else:
                scale = small.tile([H, 1], F32, tag="scale", name=f"dsc0_{rep}")
                nc.vector.memset(scale[:], 1.0)
                bias2 = small.tile([H, 1], F32, tag="bias2", name=f"db0_{rep}")
                nc.vector.memset(bias2[:], 0.0)

            for b in range(NBLK):
                bs = bs_of(b)
                aggT_ps = transpose(aggb[:, b * H : (b + 1) * H], 128, H, f"aT0{b}")
                h1T = stage.tile([H, 128], F32, tag="hnT", name=f"h1T{b}")
                nc.scalar.activation(
                    out=h1T[:], in_=aggT_ps[:], func=AF.Relu, scale=scale[:], bias=bias2[:]
                )
                hn_ps = transpose(h1T[:], H, 128, f"hn0{b}")
                build_P(b, hn_ps, False, rep)

            # ---- layers 1..L-1 ----
            if STAGE < 5:
                continue
            for layer in range(1, L):
                last = layer == L - 1
                w_l = wrest[:, (layer - 1) * H : layer * H]
                nc.gpsimd.collective_compute(
                    "AllGather", OP.bypass, replica_groups=RG,
                    ins=[P_local[:]], outs=[P_full[:]],
                )

                def epi_a(b, acc, layer=layer):
                    bs = bs_of(b)
                    hsl = hbuf[:, b * H : (b + 1) * H]
                    comb = stage.tile([128, H2], F32, tag="comb", name=f"cb{layer}_{b}")
                    nc.vector.tensor_scalar(
                        out=comb[:, H:], in0=acc[:, :H],
                        scalar1=rindeg[:, b : b + 1], scalar2=None, op0=OP.mult,
                    )
                    nc.vector.tensor_copy(out=comb[:, :H], in_=hsl)
                    nc.vector.tensor_copy(
                        out=sonh[:, b * H : (b + 1) * H], in_=acc[:, H:]
                    )
                    cT_ps = transpose(comb[:], 128, H2, f"cT{layer}_{b}")
                    cT = stage.tile([H2, 128], F32, tag="combT", name=f"cTs{layer}_{b}")
                    nc.vector.tensor_copy(out=cT[:], in_=cT_ps[:])
                    a1p = pmain.tile([H, 128], F32, tag="mm", space="PSUM", name=f"a1p{layer}_{b}")
                    nc.tensor.matmul(out=a1p[:], lhsT=watt1[:], rhs=cT[:], start=True, stop=True)
                    a1 = stage.tile([H, 128], F32, tag="a1", name=f"a1{layer}_{b}")
                    nc.scalar.activation(out=a1[:], in_=a1p[:], func=AF.Relu, bias=batt1[:])
                    a2p = pmain.tile([1, 128], F32, tag="mm", space="PSUM", name=f"a2p{layer}_{b}")
                    nc.tensor.matmul(out=a2p[:], lhsT=watt2[:], rhs=a1[:], start=True, stop=True)
                    a2 = stage.tile([1, 128], F32, tag="a2", name=f"a2{layer}_{b}")
                    nc.scalar.activation(out=a2[:], in_=a2p[:], func=AF.Sigmoid, bias=batt2[:])
                    aNp = pmain.tile([128, 1], F32, tag="mm", space="PSUM", name=f"aNp{layer}_{b}")
                    nc.tensor.matmul(out=aNp[:], lhsT=a2[:], rhs=ident[:1, :1], start=True, stop=True)
                    aN = stage.tile([128, 1], F32, tag="aN", name=f"aN{layer}_{b}")
                    nc.vector.tensor_copy(out=aN[:], in_=aNp[:])
                    anb = stage.tile([128, H], F32, tag="anb", name=f"anb{layer}_{b}")
                    nc.vector.tensor_scalar(
                        out=anb[:], in0=comb[:, H:], scalar1=aN[:], scalar2=None, op0=OP.mult
                    )
                    nc.vector.tensor_tensor(out=hsl, in0=hsl, in1=anb[:], op=OP.add)
                    g2 = stage.tile([128, H], F32, tag="g0", name=f"g2{layer}_{b}")
                    nc.vector.tensor_scalar(
                        out=g2[:], in0=anb[:], scalar1=outnorm[:, b : b + 1], scalar2=None, op0=OP.mult
                    )
                    nc.sync.dma_start(
                        out=g_local[b * BLK : b * BLK + bs, :], in_=g2[:bs, :]
                    )

                spmm(P_full, H2, epi_a, rep)
                if STAGE < 6:
                    break
                nc.gpsimd.collective_compute(
                    "AllGather", OP.bypass, replica_groups=RG,
                    ins=[g_local[:]], outs=[g_full[:]],
                )

                cov_ps = sumv_ps = None
                if SPMM_MODE == "full":
                    cov_ps = pstat.tile([H, H], F32, tag="cov", space="PSUM", name=f"cv{layer}_{rep}")
                    sumv_ps = pstat.tile([H, 1], F32, tag="sumv", space="PSUM", name=f"sv{layer}_{rep}")

                def epi_b(b, acc, layer=layer):
                    asl = aggb[:, b * H : (b + 1) * H]
                    tt = stage.tile([128, H], F32, tag="sq", name=f"tt{layer}_{b}")
                    nc.vector.tensor_tensor(
                        out=tt[:], in0=acc[:], in1=sonh[:, b * H : (b + 1) * H], op=OP.add
                    )
                    nc.vector.tensor_scalar(
                        out=asl, in0=tt[:], scalar1=innorm[:, b : b + 1], scalar2=None, op0=OP.mult
                    )
                    nc.tensor.matmul(
                        out=cov_ps[:], lhsT=asl, rhs=asl, start=(b == 0), stop=(b == NBLK - 1)
                    )
                    nc.tensor.matmul(
                        out=sumv_ps[:], lhsT=asl, rhs=ones[:], start=(b == 0), stop=(b == NBLK - 1)
                    )

                spmm(g_full, H, epi_b, rep)
                if STAGE < 7:
                    break
                if SPMM_MODE == "full":
                    scale, bias2 = stats_to_scale_bias(layer, sumv_ps, cov_ps, None, rep)
                else:
                    scale = small.tile([H, 1], F32, tag="scale", name=f"dsc{layer}_{rep}")
                    nc.vector.memset(scale[:], 1.0)
                    bias2 = small.tile([H, 1], F32, tag="bias2", name=f"db{layer}_{rep}")
                    nc.vector.memset(bias2[:], 0.0)

                for b in range(NBLK):
                    bs = bs_of(b)
                    aggT_ps = transpose(aggb[:, b * H : (b + 1) * H], 128, H, f"aT{layer}_{b}")
                    aggT = stage.tile([H, 128], F32, tag="aggT", name=f"aTs{layer}_{b}")
                    nc.vector.tensor_copy(out=aggT[:], in_=aggT_ps[:])
                    linp = pmain.tile([H, 128], F32, tag="mm", space="PSUM", name=f"lp{layer}_{b}")
                    nc.tensor.matmul(out=linp[:], lhsT=w_l, rhs=aggT[:], start=True, stop=True)
                    t2 = stage.tile([H, 128], F32, tag="t2", name=f"t2{layer}_{b}")
                    nc.vector.tensor_scalar(
                        out=t2[:], in0=linp[:], scalar1=scale[:], scalar2=bias2[:],
                        op0=OP.mult, op1=OP.add,
                    )
                    hpT_ps = transpose(hbuf[:, b * H : (b + 1) * H], 128, H, f"hpT{layer}_{b}")
                    t3 = stage.tile([H, 128], F32, tag="t3", name=f"t3{layer}_{b}")
                    nc.vector.tensor_tensor(out=t3[:], in0=t2[:], in1=hpT_ps[:], op=OP.add)
                    hnT = stage.tile([H, 128], F32, tag="hnT", name=f"hnT{layer}_{b}")
                    nc.scalar.activation(out=hnT[:], in_=t3[:], func=AF.Relu)
                    if last:
                        op_ = pmain.tile([C, 128], F32, tag="mm", space="PSUM", name=f"op{layer}_{b}")
                        nc.tensor.matmul(out=op_[:], lhsT=wfc[:], rhs=hnT[:], start=True, stop=True)
                        ot = stage.tile([C, 128], F32, tag="ot", name=f"ot{layer}_{b}")
                        nc.scalar.activation(out=ot[:], in_=op_[:], func=AF.Identity, bias=bfc[:])
                        nc.sync.dma_start(
                            out=outT_d[:, b * BLK : b * BLK + bs], in_=ot[:, :bs]
                        )
                    else:
                        hn_ps = transpose(hnT[:], H, 128, f"hn{layer}_{b}")
                        build_P(b, hn_ps, False, rep)

        ctx.close()

    return nc


# --------------------------------------------------------------------------
def _make_in_maps(meta, inputs, IN, H, C, L):
    N, S = meta["N"], meta["S"]
    f = lambda x: np.ascontiguousarray(np.asarray(x, dtype=np.float32))
    feats = f(inputs["features"])
    W_rest = f(inputs["W_rest"])
    b_rest = f(inputs["b_rest"])
    bvec = np.concatenate([f(inputs["b0"])[None, :], b_rest], axis=0).T.copy()  # [H, L]
    iota = np.tile(np.arange(128, dtype=np.float32)[None, :], (128, 1))
    ident = np.eye(128, dtype=np.float32)
    ones = np.ones((128, 1), np.float32)
    shared = dict(
        iota=iota,
        ident=ident,
        ones=ones,
        wenc1=f(inputs["enc_W1"]),
        benc1=f(inputs["enc_b1"])[:, None],
        wenc2=f(inputs["enc_W2"]),
        benc2=f(inputs["enc_b2"])[:, None],
        wenc3=f(inputs["enc_W3"]),
        benc3=f(inputs["enc_b3"])[:, None],
        w0=f(inputs["W0"]),
        bvec=bvec,
        wrest=np.ascontiguousarray(W_rest.transpose(1, 0, 2).reshape(W_rest.shape[1], -1)),
        gam=np.ascontiguousarray(f(inputs["gamma"]).T),
        bet=np.ascontiguousarray(f(inputs["beta"]).T),
        watt1=f(inputs["att_W1"]),
        batt1=f(inputs["att_b1"])[:, None],
        watt2=f(inputs["att_W2"]),
        batt2=f(inputs["att_b2"])[:, None],
        wfc=f(inputs["fc_W"]),
        bfc=f(inputs["fc_b"])[:, None],
    )
    in_maps = []
    for c in range(NCORES):
        m = dict(shared)
        m["featT"] = np.ascontiguousarray(feats[c * S : (c + 1) * S].T)
        m["idx"] = np.ascontiguousarray(meta["idx"][c])
        m["seg"] = np.ascontiguousarray(meta["seg"][c])
        m["innorm"] = np.ascontiguousarray(meta["in_norm_b"][c])
        m["outnorm"] = np.ascontiguousarray(meta["out_norm_b"][c])
        m["rindeg"] = np.ascontiguousarray(meta["r_indeg_b"][c])
        in_maps.append(m)
    return in_maps


def _prep_all(inputs, reps=1):
    feats = np.asarray(inputs["features"])
    N, IN = feats.shape
    H = np.asarray(inputs["W0"]).shape[1]
    C = np.asarray(inputs["fc_W"]).shape[1]
    L = np.asarray(inputs["gamma"]).shape[0]
    src = np.asarray(inputs["src"]).astype(np.int64)
    dst = np.asarray(inputs["dst"]).astype(np.int64)
    meta = _preprocess(N, src, dst)
    nc = _build_program(meta, IN, H, C, L, reps=reps)
    nc.compile()
    _split_excess_waits(nc)
    in_maps = _make_in_maps(meta, inputs, IN, H, C, L)
    return meta, nc, in_maps, (IN, H, C, L)


def kernel(**inputs):
    meta, nc, in_maps, (IN, H, C, L) = _prep_all(inputs, reps=1)
    res = run_bass_kernel_spmd(nc, in_maps, list(range(NCORES)))
    S, N = meta["S"], meta["N"]
    out = np.empty((N, C), np.float32)
    for c in range(NCORES):
        out[c * S : (c + 1) * S] = res.results[c]["outT"].T
    return out

